# revision 21
# baseline (speedup 1.0000x reference)
"""Causal single-head attention (B=4, T=4096, C=1024, H=128) on 8 Trainium2
NeuronCores — bf16, pipelined-startup edition.

Sharding: 8 cores = 4 batches x 2 key-parity shards: each core handles one
batch and the keys in every other 128-block (parity h = core % 2), computing
UN-normalized partial attention (numerator O^T and denominator) for ALL 4096
queries of its batch. The host passes x^T with columns block-permuted so this
core's keys sit at the EVEN 128-block positions, plus a pre-gathered x^T
restricted to those key columns (xkv), plus the two diagonal-block mask
tiles, plus weights already in the on-chip [128, c*128] layout so their DMA
is contiguous. Host un-permutes the query axis and combines:
out = (O0 + O1) / (d0 + d1).

Numerics: bf16 operands everywhere on the PE (1 cycle/row, FWL hides
LDWEIGHTS), fp32 PSUM accumulation. Measured end-to-end rel err ~5e-3.
fp8 was tried and rejected: each fp8 quantization on the value path alone
contributes 2-4e-2 relative error, busting the 2e-2 budget.

Pipelining:
  - x streams on the gpsimd DMA queue, weights/masks/outputs on the sync
    queue, so input streaming and output drain don't serialize.
  - Supertile 0's x tiles arrive as per-chunk DMAs so the first projection
    matmul only waits for chunk 0, not the whole megatile.
  - ~24 tiny warmup matmuls run during the DMA fill to lift the PE out of
    its cold HAM state before real work arrives.
  - Per query group j (512 queries, nk=2(j+1) key blocks): S^T pair ->
    PSUM [128,1024]; exp via one ACT activation -> bf16 SBUF with softmax
    scale + bias -3 folded in; diagonal pair masked by one DVE multiply;
    PV lags TWO pairs behind S/exp (the last two pairs spill into the next
    group) so the PE never waits on ACT; denominator accumulates on the DVE
    (all-bf16 2x mode) and reduces with two ones-matmuls per group.
"""

import sys
import numpy as np

sys.path.insert(0, "/opt/trn_rl_repo")

B, T, C, H = 4, 4096, 1024, 128
KB = 128            # key block
QG = 512            # query group
NKB = T // KB       # 32 global key blocks
NQG = T // QG       # 8 query groups
NCH = C // 128      # 8 contraction chunks
NST = 4             # supertiles of 1024 positions
SCALE = float(H) ** -0.5
EXPBIAS = -3.0

_prog_cache = {}


def _build_program():
    import concourse.mybir as mybir
    import concourse.tile as tile
    from concourse import bacc

    F32 = mybir.dt.float32
    BF16 = mybir.dt.bfloat16
    AF = mybir.ActivationFunctionType

    nc = bacc.Bacc()
    xt = nc.dram_tensor("xt", [128, NST * NCH * 1024], BF16,
                        kind="ExternalInput")
    wq = nc.dram_tensor("wq", [128, NCH * 128], BF16, kind="ExternalInput")
    wk = nc.dram_tensor("wk", [128, NCH * 128], BF16, kind="ExternalInput")
    wv = nc.dram_tensor("wv", [128, NCH * 128], BF16, kind="ExternalInput")
    mp = nc.dram_tensor("mp", [KB, 2 * QG], BF16, kind="ExternalInput")
    idb = nc.dram_tensor("idb", [128, 128], BF16, kind="ExternalInput")
    ot = nc.dram_tensor("ot", [H, T], BF16, kind="ExternalOutput")
    dn = nc.dram_tensor("dn", [1, T], F32, kind="ExternalOutput")

    with tile.TileContext(nc) as tc:
        with (
            tc.tile_pool(name="singles", bufs=1) as singles,
            tc.tile_pool(name="persist", bufs=1) as persist,
            tc.tile_pool(name="xqp", bufs=4) as xqp,
            tc.tile_pool(name="epool", bufs=6) as epool,
            tc.tile_pool(name="eacc", bufs=2) as eaccp,
            tc.tile_pool(name="vstage", bufs=2) as vstagep,
            tc.tile_pool(name="outs", bufs=4) as outsp,
            tc.tile_pool(name="spair", bufs=2, space="PSUM") as spairp,
            tc.tile_pool(name="pop", bufs=1, space="PSUM") as pop,
            tc.tile_pool(name="pproj", bufs=2, space="PSUM") as pproj,
        ):
            # ---- constants (DVE memsets run during the engine preamble) ----
            scratch = singles.tile([128, QG], BF16, tag="scratch")
            nc.vector.memset(scratch, 0.125)
            ones_m = singles.tile([128, 128], BF16, tag="ones_m")
            nc.vector.memset(ones_m, 1.0)
            ebias = singles.tile([128, 1], F32, tag="ebias")
            nc.vector.memset(ebias, EXPBIAS)

            # ---- PE warmup: lift HAM out of the cold state during DMA fill
            wps = pop.tile([128, QG], F32, tag="dpsum", name="wpsum")
            for _ in range(5):
                nc.tensor.matmul(wps, lhsT=scratch[:, 0:128],
                                 rhs=scratch, start=True, stop=True)

            # ---- weights first on the sync queue (K proj blocks on wk);
            # masks/identity aren't consumed until ~16us in ----
            w_sb = {}
            for name, w in (("wk", wk), ("wv", wv), ("wq", wq)):
                t_ = singles.tile([128, NCH * 128], BF16, tag=f"w_{name}")
                nc.sync.dma_start(out=t_, in_=w[:])
                w_sb[name] = t_.rearrange("p (c h) -> p c h", c=NCH)
            identb = singles.tile([128, 128], BF16, tag="identb")
            nc.sync.dma_start(out=identb, in_=idb[:])
            mp_sb = singles.tile([KB, 2 * QG], BF16, tag="mp")
            nc.sync.dma_start(out=mp_sb, in_=mp[:])

            qT = persist.tile([128, T], BF16, tag="qT")
            kT = persist.tile([128, T // 2], BF16, tag="kT")
            v_sb = persist.tile([128, T // 2], BF16, tag="v")

            def stream_x(s):
                xq = xqp.tile([128, NCH * 1024], BF16, tag="xq")
                base = s * NCH * 1024
                if s == 0:
                    for h2 in range(2):
                        nc.gpsimd.dma_start(
                            out=xq.rearrange("p (c t) -> p c t", c=NCH)
                            [:, :, h2 * 512:(h2 + 1) * 512],
                            in_=xt[:, base:base + NCH * 1024]
                            .rearrange("p (c t) -> p c t", c=NCH)
                            [:, :, h2 * 512:(h2 + 1) * 512])
                else:
                    nc.gpsimd.dma_start(
                        out=xq, in_=xt[:, base:base + NCH * 1024])
                # full view and even-key-block view (K/V read the latter)
                return (xq.rearrange("p (c t) -> p c t", c=NCH),
                        xq.rearrange("p (c u par b) -> p c u par b",
                                     c=NCH, u=4, par=2, b=128))

            def proj_steps(s, xqv, xkvv):
                def kproj():
                    kp = pproj.tile([128, QG], F32, tag="proj", name="kp")
                    for c in range(NCH):
                        nc.tensor.matmul(
                            kp, lhsT=w_sb["wk"][:, c, :],
                            rhs=xkvv[:, c, :, 0, :],
                            start=(c == 0), stop=(c == NCH - 1))
                    nc.vector.tensor_copy(kT[:, s * QG:(s + 1) * QG], kp)

                vstage = vstagep.tile([128, QG], BF16, tag="vstage")

                def vproj():
                    vp = pproj.tile([128, QG], F32, tag="proj", name="vp")
                    for c in range(NCH):
                        nc.tensor.matmul(
                            vp, lhsT=w_sb["wv"][:, c, :],
                            rhs=xkvv[:, c, :, 0, :],
                            start=(c == 0), stop=(c == NCH - 1))
                    nc.vector.tensor_copy(vstage, vp)

                def qproj(half):
                    def f():
                        qp = pproj.tile([128, QG], F32, tag="proj", name="qp")
                        for c in range(NCH):
                            nc.tensor.matmul(
                                qp, lhsT=w_sb["wq"][:, c, :],
                                rhs=xqv[:, c, half * QG:(half + 1) * QG],
                                start=(c == 0), stop=(c == NCH - 1))
                        nc.vector.tensor_copy(
                            qT[:, (2 * s + half) * QG:
                               (2 * s + half + 1) * QG], qp)
                    return f

                def vtrans():
                    for u in range(4):
                        tp = pproj.tile([128, 128], BF16, tag="proj",
                                        name="tp")
                        nc.tensor.transpose(
                            tp, vstage[:, u * 128:(u + 1) * 128], identb)
                        nc.vector.tensor_copy(
                            v_sb[:, (4 * s + u) * 128:(4 * s + u + 1) * 128],
                            tp)

                return [kproj, vproj, qproj(0), qproj(1), vtrans]

            def proj_steps0(xqv, xkvv):
                vstage = vstagep.tile([128, QG], BF16, tag="vstage",
                                      name="vstage0")

                def half(h2):
                    def kv(which, dst):
                        def f():
                            pp = pproj.tile([128, 256], F32, tag="proj",
                                            name="pp0")
                            for c in range(NCH):
                                nc.tensor.matmul(
                                    pp, lhsT=w_sb[which][:, c, :],
                                    rhs=xkvv[:, c, 2 * h2:2 * h2 + 2, 0, :],
                                    start=(c == 0), stop=(c == NCH - 1))
                            nc.vector.tensor_copy(dst, pp)
                        return f

                    def qp():
                        qp_ = pproj.tile([128, QG], F32, tag="proj",
                                         name="qp0")
                        for c in range(NCH):
                            nc.tensor.matmul(
                                qp_, lhsT=w_sb["wq"][:, c, :],
                                rhs=xqv[:, c, h2 * QG:(h2 + 1) * QG],
                                start=(c == 0), stop=(c == NCH - 1))
                        nc.vector.tensor_copy(
                            qT[:, h2 * QG:(h2 + 1) * QG], qp_)

                    def vt():
                        for u in (2 * h2, 2 * h2 + 1):
                            tp = pproj.tile([128, 128], BF16, tag="proj",
                                            name="tp0")
                            nc.tensor.transpose(
                                tp, vstage[:, u * 128:(u + 1) * 128], identb)
                            nc.vector.tensor_copy(
                                v_sb[:, u * 128:(u + 1) * 128], tp)

                    return [kv("wk", kT[:, h2 * 256:(h2 + 1) * 256]),
                            kv("wv", vstage[:, h2 * 256:(h2 + 1) * 256]),
                            qp, vt]

                return half(0), half(1)

            state = {}

            def pv(es, p, start, stop, diag=False):
                if start:
                    state["o"] = pop.tile([128, QG], F32, tag="opsum",
                                          name="opsum")
                nc.tensor.matmul(
                    state["o"],
                    lhsT=v_sb[:, 2 * p * KB:(2 * p + 1) * KB],
                    rhs=es[p][:, 0:QG], start=start, stop=False)
                blk = 2 * p + 1
                if diag:
                    # block nk-1 is dead outside query cols [256,512)
                    nc.tensor.matmul(
                        state["o"][:, 256:QG],
                        lhsT=v_sb[:, blk * KB:(blk + 1) * KB],
                        rhs=es[p][:, QG + 256:2 * QG],
                        start=False, stop=stop)
                else:
                    nc.tensor.matmul(
                        state["o"],
                        lhsT=v_sb[:, blk * KB:(blk + 1) * KB],
                        rhs=es[p][:, QG:2 * QG], start=False, stop=stop)

            def attention_group(j, pending, interleave=()):
                npair = j + 1
                qrhs = qT[:, j * QG:(j + 1) * QG]
                es = []
                acc = eaccp.tile([KB, 2 * QG], BF16, tag="eacc", name="eacc")
                inter = list(interleave)
                for p in range(npair):
                    diag = p == npair - 1
                    sp = spairp.tile([KB, 2 * QG], F32, tag="spair",
                                     name="sp")
                    nc.tensor.matmul(
                        sp[:, 0:QG],
                        lhsT=kT[:, 2 * p * KB:(2 * p + 1) * KB],
                        rhs=qrhs, start=True, stop=True)
                    if diag:
                        nc.tensor.matmul(
                            sp[:, QG + 256:2 * QG],
                            lhsT=kT[:, (2 * p + 1) * KB:(2 * p + 2) * KB],
                            rhs=qrhs[:, 256:QG], start=True, stop=True)
                    else:
                        nc.tensor.matmul(
                            sp[:, QG:2 * QG],
                            lhsT=kT[:, (2 * p + 1) * KB:(2 * p + 2) * KB],
                            rhs=qrhs, start=True, stop=True)
                    if p == 0 and pending is not None:
                        pending()
                    elif p >= 1 and inter:
                        inter.pop(0)()
                    ep = epool.tile([KB, 2 * QG], BF16, tag="e", name="ep")
                    if diag:
                        nc.scalar.activation(ep[:, 0:QG], sp[:, 0:QG],
                                             AF.Exp, bias=ebias, scale=SCALE)
                        nc.scalar.activation(
                            ep[:, QG + 256:2 * QG], sp[:, QG + 256:2 * QG],
                            AF.Exp, bias=ebias, scale=SCALE)
                        nc.vector.tensor_mul(ep[:, 0:QG], ep[:, 0:QG],
                                             mp_sb[:, 0:QG])
                        nc.vector.tensor_mul(
                            ep[:, QG + 256:2 * QG], ep[:, QG + 256:2 * QG],
                            mp_sb[:, QG + 256:2 * QG])
                        if p == 0:
                            nc.vector.tensor_copy(acc[:, 0:QG], ep[:, 0:QG])
                            nc.vector.memset(acc[:, QG:QG + 256], 0.0)
                            nc.vector.tensor_copy(
                                acc[:, QG + 256:2 * QG],
                                ep[:, QG + 256:2 * QG])
                        else:
                            nc.vector.tensor_add(acc[:, 0:QG], acc[:, 0:QG],
                                                 ep[:, 0:QG])
                            nc.vector.tensor_add(
                                acc[:, QG + 256:2 * QG],
                                acc[:, QG + 256:2 * QG],
                                ep[:, QG + 256:2 * QG])
                    else:
                        nc.scalar.activation(ep, sp, AF.Exp,
                                             bias=ebias, scale=SCALE)
                        if p == 0:
                            nc.vector.tensor_copy(acc, ep)
                        else:
                            nc.vector.tensor_add(acc, acc, ep)
                    es.append(ep)
                    lag = 1 if j == NQG - 1 else 2
                    if p >= lag:
                        pv(es, p - lag, start=(p == lag), stop=False)
                for f in inter:
                    f()

                def flush():
                    lag = 1 if j == NQG - 1 else 2
                    for pp in range(max(npair - lag, 0), npair):
                        pv(es, pp, start=(pp == 0), stop=(pp == npair - 1),
                           diag=(pp == npair - 1))
                    accf = eaccp.tile([KB, QG], BF16, tag="eaccf",
                                      name="accf")
                    nc.vector.tensor_add(accf, acc[:, 0:QG],
                                         acc[:, QG:2 * QG])
                    dps = pop.tile([128, QG], F32, tag="dpsum",
                                   name="dpsum")
                    nc.tensor.matmul(dps, lhsT=ones_m, rhs=accf,
                                     start=True, stop=True)
                    osb = outsp.tile([128, QG], BF16, tag="osb")
                    nc.vector.tensor_copy(osb, state["o"])
                    nc.sync.dma_start(out=ot[:, j * QG:(j + 1) * QG], in_=osb)
                    dsb = outsp.tile([1, QG], F32, tag="dsb")
                    nc.vector.tensor_copy(dsb, dps[0:1, :])
                    nc.sync.dma_start(out=dn[:, j * QG:(j + 1) * QG], in_=dsb)
                return flush

            pending = None
            xqv, xkvv = stream_x(0)
            h0, h1 = proj_steps0(xqv, xkvv)
            for f in h0:
                f()
            pending = attention_group(0, pending)
            for f in h1:
                f()
            nxq, nxkv = stream_x(1)
            nxt = proj_steps(1, nxq, nxkv)
            pending = attention_group(1, pending, interleave=nxt)
            for s in range(1, NST):
                if s + 1 < NST:
                    nxq, nxkv = stream_x(s + 1)
                    nxt = proj_steps(s + 1, nxq, nxkv)
                else:
                    nxt = []
                pending = attention_group(2 * s, pending)
                pending = attention_group(2 * s + 1, pending, interleave=nxt)
            pending()

    nc.finalize()
    return nc


def _get_program():
    if "nc" not in _prog_cache:
        _prog_cache["nc"] = _build_program()
    return _prog_cache["nc"]


def _to_bf16(a):
    import ml_dtypes
    return np.asarray(a, np.float32).astype(ml_dtypes.bfloat16)


def _warr(w):
    # host-side rearrange to the on-chip [128, c*128] layout (contiguous DMA)
    return np.ascontiguousarray(
        _to_bf16(w).reshape(NCH, 128, H).transpose(1, 0, 2).reshape(128, -1))


def _host_prepare(x, Wq, Wk, Wv):
    """Per-core inputs. Core c: batch b=c//2, parity h=c%2."""
    w16 = {n: _warr(w) for n, w in (("wq", Wq), ("wk", Wk), ("wv", Wv))}
    per_core = []
    for c in range(8):
        b, h = c // 2, c % 2
        pos2glob = np.arange(NKB)
        if h == 1:
            pos2glob = pos2glob.reshape(-1, 2)[:, ::-1].reshape(-1)
        perm = (pos2glob[:, None] * KB + np.arange(KB)[None, :]).reshape(-1)
        xtb = _to_bf16(x[b].T[:, perm])
        # [c*128+p, s*1024+t] -> [p, (s c t)] so supertile DMAs are contiguous
        xt2 = np.ascontiguousarray(
            xtb.reshape(NCH, 128, NST, 1024).transpose(1, 2, 0, 3)
            .reshape(128, NST * NCH * 1024))
        sub = np.arange(QG) // KB
        off = np.arange(QG) % KB
        glob_sub = sub if h == 0 else (sub ^ 1)
        qoff = glob_sub * KB + off
        kk = np.arange(KB)[:, None]
        m0 = (qoff[None, :] >= kk + h * KB).astype(np.float32)
        m1 = (qoff[None, :] >= kk + h * KB + 256).astype(np.float32)
        per_core.append(dict(perm=perm, in_map={
            "xt": xt2,
            "wq": w16["wq"], "wk": w16["wk"], "wv": w16["wv"],
            "mp": _to_bf16(np.concatenate([m0, m1], axis=1)),
            "idb": _to_bf16(np.eye(128, dtype=np.float32)),
        }))
    return per_core


def run(x, Wq, Wk, Wv, trace=False):
    from concourse.bass_utils import run_bass_kernel_spmd

    x = np.asarray(x, np.float32)
    nc = _get_program()
    per_core = _host_prepare(x, Wq, Wk, Wv)
    res = run_bass_kernel_spmd(
        nc, [pc["in_map"] for pc in per_core], core_ids=list(range(8)),
        trace=trace,
    )
    out = np.zeros((B, T, H), np.float32)
    for b in range(B):
        num = np.zeros((H, T), np.float64)
        den = np.zeros((1, T), np.float64)
        for c in (2 * b, 2 * b + 1):
            inv = np.argsort(per_core[c]["perm"])
            num += np.asarray(res.results[c]["ot"], np.float32)[:, inv]
            den += res.results[c]["dn"][:, inv]
        out[b] = (num / den).T
    return out, res


def kernel(x, Wq, Wk, Wv):
    out, _ = run(x, Wq, Wk, Wv, trace=False)
    return out
